# revision 30
# baseline (speedup 1.0000x reference)
"""Trainium2 Bass kernel for single-head attention (B=4, S=4096, C=D=512).

Sharding: 8 cores = 4 batches x 2 query-halves. Each core receives xT
([C, S], host-pre-transposed bf16) ROLLED so its query half occupies
columns 0..2047 (attention over keys is order-invariant, so rolling keys
is exact).

The key trick: both weight pairs fold on the host, eliminating two of the
four projection stages on-chip (exact algebra, not an approximation):

  score_qk = (x_q Wq + bq)(x_k Wk + bk)^T
           = [x_q (Wq Wk^T) + bq Wk^T] x_k^T + const(q)   [cancels in softmax]
  out      = (P/l)(x Wv + bv) Wo + bo
           = (P/l) x (Wv Wo) + (bv Wo + bo)

With M = Wq Wk^T and N = Wv Wo precomputed f32 on the host:
  - NO K projection and NO kT tile: score matmuls contract q~ = x M + bq Wk^T
    directly against the resident xT chunks,
  - NO output projection: the attention accumulation P^T-slices x (x N)
    is issued with lhsT = pT column slices so PSUM accumulates at[q, d],
    already output-oriented; out rows = at * (1/l) + (bv Wo + bo),
  - the bias fold keeps everything exact: bq enters via the q~ copy's
    bias port (b~ = bq Wk^T), bk cancels in softmax, bv+bo are added on
    the host after gather.

Per-core PE work drops from ~302us to ~261us of matmul streaming.

On-chip layout notes:
  - x is transposed AND tiled on the HOST (x[rg, dc, p, s]) so every
    (rg, dc) chunk is one dense 128KB DMA and the DMA'd tile is itself
    the persistent matmul operand for V~ projection, scores, and q~.
  - M/N are bf16 from the host, DMA'd straight into persistent tiles.
  - Scores are computed transposed (scoreT[s, q]) so exp(scoreT) feeds
    the attention matmul directly with no per-tile transposes.
  - Row sums l[q] accumulate on the DVE (l_sb += pT per key tile); 1/l
    is applied per query row via a per-partition scale AP.
  - PSUM->SBUF copies for vv/qT run on the Scalar engine.
  - The s-loop is software-pipelined: score matmuls for key-tile st+1/st+2
    are issued before the exp(st)-consuming matmuls so the in-order PE
    never waits on the ScalarE.
"""

import sys

for _p in ("/opt/trn_rl_repo", "/root/.axon_site/_ro/trn_rl_repo"):
    if _p not in sys.path:
        sys.path.append(_p)

import numpy as np
import ml_dtypes
import concourse.bacc as bacc
import concourse.mybir as mybir
import concourse.tile as tile
from concourse.bass_utils import run_bass_kernel_spmd

F32 = mybir.dt.float32
BF16 = mybir.dt.bfloat16

MM_DT = BF16

B, S, C, D = 4, 4096, 512, 512
Q = S // 2          # queries per core
N_CORES = 8
SCALE = float(D) ** -0.5
QB = 512            # query block (psum bank width in fp32)
N_QB = Q // QB      # 4 query blocks per core
N_ST = S // 128     # 32 key tiles
N_DC = C // 128     # 4 contraction chunks
N_RG = S // 512     # 8 row groups


def _build_program():
    nc = bacc.Bacc(None, target_bir_lowering=False, debug=False)

    # host-transposed AND host-tiled: x[rg, dc, p, s] = xT[dc*128+p, rg*512+s],
    # so every (rg, dc) chunk is one dense 128KB DMA
    x = nc.dram_tensor("x", [N_RG, N_DC, 128, 512], BF16, kind="ExternalInput")
    w_dram = {
        name: nc.dram_tensor(name, [C, D], BF16, kind="ExternalInput")
        for name in ("M", "N")
    }
    bq_dram = nc.dram_tensor("bq", [D], F32, kind="ExternalInput")  # bq Wk^T
    # bf16 output: halves output DMA traffic; adds <= 2^-8 relative rounding
    # on top of 3.4e-3 absmax err, keeping the metric ~1e-2 << 2e-2
    out = nc.dram_tensor("out", [Q, D], BF16, kind="ExternalOutput")

    ActFn = mybir.ActivationFunctionType

    with tile.TileContext(nc) as tc:
        persist = tc.alloc_tile_pool(name="persist", bufs=1)
        const = tc.alloc_tile_pool(name="const", bufs=1)

        ones_f32 = const.tile([128, 128], F32, tag="ones_f32")
        nc.vector.memset(ones_f32[:], 1.0)

        wts = {}

        def emit_weight(name, engine):
            wt = persist.tile([128, N_DC, D], MM_DT, tag=f"w_{name}", name=f"w_{name}")
            for dc in range(N_DC):
                engine.dma_start(wt[:, dc, :], w_dram[name][dc * 128 : (dc + 1) * 128, :])
            wts[name] = wt

        # ---- persistent activations ----
        vv = persist.tile([128, N_ST, D], MM_DT, tag="v")   # vv[p, i, e] = (x N)[i*128+p, e]
        xT = persist.tile([128, N_DC, S], MM_DT, tag="xT")  # xT[p, dc, s] = x[s, dc*128+p]

        # ================= phase A: V~ = x N projection =================
        ps_proj = tc.alloc_tile_pool(name="ps_proj", bufs=4, space="PSUM")

        def emit_xdma(rg, eng):
            for dc in range(N_DC):
                eng.dma_start(xT[:, dc, rg * 512 : (rg + 1) * 512], x[rg, dc])

        # rg0/rg1 x chunks on sync, N chunks on scalar: the two queues
        # deliver the dc-k chunk pairs in lockstep with rg0's dc-outer
        # matmuls; M (needed only at phase-B start) follows on scalar.
        emit_xdma(0, nc.sync)
        emit_weight("N", nc.scalar)
        emit_xdma(1, nc.sync)

        warm = const.tile([1, 1], F32, tag="warm")
        nc.scalar.activation(warm[:], ones_f32[0:1, 0:1], ActFn.Exp, scale=1.0)

        ones_bf = const.tile([128, 1], MM_DT, tag="ones_bf")
        nc.vector.memset(ones_bf[:], 1.0)
        bqT = const.tile([128, N_DC], F32, tag="bqT")

        for rg in range(N_RG):              # 8 row groups of 512 rows
            # V~ for these 512 rows.  rg0 runs dc-outer, chasing the N DMAs.
            pv = [ps_proj.tile([128, 512], F32, tag="pv", name=f"pv{rg}_{rt}")
                  for rt in range(4)]
            if rg == 0:
                for dc in range(N_DC):
                    for rt in range(4):
                        nc.tensor.matmul(pv[rt][:], xT[:, dc, rg * 512 + rt * 128 : rg * 512 + (rt + 1) * 128],
                                         wts["N"][:, dc, :], start=(dc == 0), stop=(dc == N_DC - 1))
            else:
                for rt in range(4):
                    for dc in range(N_DC):
                        nc.tensor.matmul(pv[rt][:], xT[:, dc, rg * 512 + rt * 128 : rg * 512 + (rt + 1) * 128],
                                         wts["N"][:, dc, :], start=(dc == 0), stop=(dc == N_DC - 1))
            for rt in range(4):
                nc.scalar.activation(vv[:, rg * 4 + rt, :], pv[rt][:], ActFn.Copy)
            # queue upcoming x row groups / weights while rg's matmuls run
            if rg == 0:
                emit_xdma(2, nc.scalar)
                emit_xdma(3, nc.sync)
            elif rg == 1:
                emit_weight("M", nc.scalar)
                emit_xdma(4, nc.sync)
            elif rg == 2:
                emit_xdma(5, nc.scalar)
                emit_xdma(6, nc.sync)
                emit_xdma(7, nc.scalar)
            elif rg == 4:
                # tiny 4B-element bias DMAs, needed only at phase-B start
                for g in range(N_DC):
                    nc.gpsimd.dma_start(bqT[:, g : g + 1],
                                        bq_dram[g * 128 : (g + 1) * 128].unsqueeze(1))

        ps_proj.release()

        # ================= phase B: attention =================
        with tc.tile_pool(name="qT", bufs=2) as qTp, \
             tc.tile_pool(name="pT", bufs=8) as pTp, \
             tc.tile_pool(name="rl", bufs=2) as rlp, \
             tc.tile_pool(name="osb", bufs=4) as osbp, \
             tc.tile_pool(name="ps_at", bufs=4, space="PSUM") as ps_atp, \
             tc.tile_pool(name="ps_s", bufs=3, space="PSUM") as ps_sp, \
             tc.tile_pool(name="ps_l", bufs=1, space="PSUM") as ps_lp:

            def emit_qproj(qb):
                # q~ = x M + bq Wk^T for one 512-query block, straight from
                # the resident xT columns [qb*512, (qb+1)*512).
                qT = qTp.tile([128, N_DC, 512], MM_DT, tag="qT", name=f"qT{qb}")
                for g in range(N_DC):
                    pq = ps_sp.tile([128, 512], F32, tag="ss", name=f"pq{qb}_{g}")
                    for dc in range(N_DC):
                        nc.tensor.matmul(pq[:], wts["M"][:, dc, g * 128 : (g + 1) * 128],
                                         xT[:, dc, qb * 512 : (qb + 1) * 512],
                                         start=(dc == 0), stop=(dc == N_DC - 1))
                    nc.scalar.activation(qT[:, g, :], pq[:], ActFn.Identity,
                                         bias=bqT[:, g : g + 1])
                return qT

            def emit_score(qb, st, qT):
                # scoreT[s in st, q] = sum_dc xT[:, dc, st]^T qT[:, dc, :]
                ss = ps_sp.tile([128, 512], F32, tag="ss", name=f"ss{qb}_{st}")
                for dc in range(N_DC):
                    nc.tensor.matmul(ss[:], xT[:, dc, st * 128 : (st + 1) * 128],
                                     qT[:, dc, :], start=(dc == 0), stop=(dc == N_DC - 1))
                return ss

            qT_cur = emit_qproj(0)
            for qb in range(N_QB):
                qT = qT_cur
                l_sb = rlp.tile([128, 512], F32, tag="l_sb", name=f"lsb{qb}")
                # at[qt][q in block, d] accumulates the UNNORMALIZED output
                # rows for this query block (P x N-projected values)
                at_ps = [ps_atp.tile([128, 512], F32, tag="at", name=f"at{qb}_{qt}")
                         for qt in range(4)]
                ss_q = [emit_score(qb, 0, qT), emit_score(qb, 1, qT)]
                for st in range(N_ST):
                    if st + 2 < N_ST:
                        ss_q.append(emit_score(qb, st + 2, qT))
                    ss = ss_q.pop(0)
                    pT = pTp.tile([128, 512], MM_DT, tag="pT", name=f"pT{qb}_{st}")
                    nc.scalar.activation(pT[:], ss[:], ActFn.Exp, scale=SCALE)
                    for qt in range(4):
                        nc.tensor.matmul(at_ps[qt][:], pT[:, qt * 128 : (qt + 1) * 128],
                                         vv[:, st, :], start=(st == 0), stop=(st == N_ST - 1))
                    # row-sum accumulation on the DVE (off the PE)
                    if st == 0:
                        nc.vector.tensor_copy(l_sb[:], pT[:])
                    else:
                        nc.vector.tensor_add(l_sb[:], l_sb[:], pT[:])

                if qb + 1 < N_QB:
                    qT_cur = emit_qproj(qb + 1)

                # --- epilogue: 1/l arranged with queries on partitions.
                # One tiny bf16 matmul per 128-query block folds the partition
                # sum AND the transpose: lt[q, 0] = sum_p lbf[p, qt*128 + q]
                # (lhsT = lbf slice, rhs = ones column) ---
                lbf = rlp.tile([128, 512], MM_DT, tag="lbf", name=f"lbf{qb}")
                nc.vector.tensor_copy(lbf[:], l_sb[:])
                lt_ps = ps_lp.tile([128, 4], F32, tag="l", name=f"lt{qb}")
                for qt in range(4):
                    nc.tensor.matmul(lt_ps[:, qt : qt + 1],
                                     lbf[:, qt * 128 : (qt + 1) * 128],
                                     ones_bf[:, 0:1])
                rlT = rlp.tile([128, 4], F32, tag="rlT", name=f"rlT{qb}")
                nc.vector.reciprocal(rlT[:], lt_ps[:])

                # out rows = at * (1/l); for the final block split the scaled
                # copies across Vector+Scalar and both DMA queues to shorten
                # the kernel tail
                last = qb == N_QB - 1
                for qt in range(4):
                    ot = osbp.tile([128, D], MM_DT, tag="ot", name=f"ot{qb}_{qt}")
                    row = out[(qb * 4 + qt) * 128 : (qb * 4 + qt + 1) * 128, :]
                    if last:
                        nc.vector.tensor_scalar_mul(ot[:, 0:256], at_ps[qt][:, 0:256],
                                                    rlT[:, qt : qt + 1])
                        nc.scalar.activation(ot[:, 256:512], at_ps[qt][:, 256:512],
                                             ActFn.Copy, scale=rlT[:, qt : qt + 1])
                        # scalar's queue is busy with the ACTIVATE halves:
                        # put 6 of the 8 half-DMAs on sync
                        eng1 = nc.scalar if qt < 2 else nc.sync
                        nc.sync.dma_start(row[:, 0:256], ot[:, 0:256])
                        eng1.dma_start(row[:, 256:512], ot[:, 256:512])
                    else:
                        nc.vector.tensor_scalar_mul(ot[:], at_ps[qt][:], rlT[:, qt : qt + 1])
                        eng = nc.sync if qt % 2 == 0 else nc.scalar
                        eng.dma_start(row, ot[:])

        const.release()
        persist.release()

    nc.compile()
    return nc


_NC_CACHE = None


def _get_nc():
    global _NC_CACHE
    if _NC_CACHE is None:
        _NC_CACHE = _build_program()
    return _NC_CACHE


def kernel(**inputs):
    f32 = np.float32
    x = np.asarray(inputs["x"], dtype=f32)
    # host-side transpose to xT[c, s] per batch, cast bf16
    xt = np.ascontiguousarray(
        x.reshape(B, S, C).transpose(0, 2, 1)).astype(ml_dtypes.bfloat16)

    def tile_x(xb):
        # xT[c, s] -> [rg, dc, p, s] so each (rg, dc) chunk is contiguous
        return np.ascontiguousarray(
            xb.reshape(N_DC, 128, N_RG, 512).transpose(2, 0, 1, 3))

    Wq = np.asarray(inputs["Wq"], dtype=f32)
    Wk = np.asarray(inputs["Wk"], dtype=f32)
    Wv = np.asarray(inputs["Wv"], dtype=f32)
    Wo = np.asarray(inputs["Wo"], dtype=f32)
    # host-side weight folds (exact algebra, f32):
    #   score = [x (Wq Wk^T) + bq Wk^T] x^T  (+ per-query const, cancels)
    #   out   = (P/l) x (Wv Wo) + (bv Wo + bo)
    M = np.ascontiguousarray(Wq @ Wk.T).astype(ml_dtypes.bfloat16)
    N = np.ascontiguousarray(Wv @ Wo).astype(ml_dtypes.bfloat16)
    bqf = np.ascontiguousarray(np.asarray(inputs["bq"], dtype=f32) @ Wk.T)
    bo_eff = np.asarray(inputs["bo"], dtype=f32) + np.asarray(inputs["bv"], dtype=f32) @ Wo

    in_maps = []
    for c in range(N_CORES):
        b, h = divmod(c, 2)
        xb = xt[b]
        if h:
            # roll keys so this core's query half occupies columns 0..2047;
            # attention over keys is order-invariant so this is exact.
            xb = np.concatenate([xb[:, Q:], xb[:, :Q]], axis=1)
        in_maps.append({"x": tile_x(xb), "bq": bqf, "M": M, "N": N})

    nc = _get_nc()
    try:
        res = run_bass_kernel_spmd(nc, in_maps, core_ids=list(range(N_CORES)))
    except Exception:
        # transient NRT/device hiccups recover on retry
        import time
        time.sleep(15)
        res = run_bass_kernel_spmd(nc, in_maps, core_ids=list(range(N_CORES)))

    out = np.empty((B, S, D), dtype=f32)
    for c in range(N_CORES):
        b, h = divmod(c, 2)
        out[b, h * Q : (h + 1) * Q] = np.asarray(res.results[c]["out"]).astype(f32)
    if np.any(bo_eff):
        out += bo_eff
    return out.reshape(B, 64, 64, D)
